# revision 3
# baseline (speedup 1.0000x reference)
"""Two-layer SimpleRNN (B=64, T=80, U=2048) on 8 TRN2 NeuronCores — v5.

Tensor-parallel like v1 (each core owns a 256-col slice of the hidden units,
weights SBUF-resident, transposed-layout matmuls), but the per-step all-gather
of the [256, 64] h-chunks goes through the runtime collective engine instead
of software-DGE remote DMA: SBUF -> DRAM write, AllGather into a rank-major
shared [NC*128, SLOT] buffer, per-slot DRAM -> SBUF reads.

v1's trace showed each SWDGE remote-DMA prep costs ~7.3us of serialized queue
latency (7 preps/step = ~51us of the 57us step); multi-dest broadcasts and
runtime-branched sends all fail on this HW path. The collective engine is the
one proven alternative (the v1 barrier uses it), needs no per-core constants
(it handles slot offsets internally), no remote semaphores, and no arrival
waits — Tile's ordinary dependency tracking orders write -> CC -> read ->
consuming matmuls.

Receiver slot s holds the chunk of core s (absolute indexing, no XOR
permutation); every core's own chunk also arrives via the CC.

kernel(**inputs) takes the FULL unsharded inputs and returns the FULL output.
"""

import sys
import time

sys.path.insert(0, "/opt/trn_rl_repo")

import numpy as np
import ml_dtypes

import concourse.bass as bass
import concourse.mybir as mybir
import concourse.bacc as bacc
import concourse.tile as tile
import concourse.bass_utils as bass_utils
from concourse.tile_rust import add_dep_helper

B = 64          # batch
import os
T = int(os.environ.get("RNN_T", "80"))  # sequence length
E = 100         # embedding dim
EP = 128        # embedding dim padded to one partition tile
U = 2048        # hidden units
NC = 8          # cores
UC = U // NC    # hidden columns per core (256)
NT = UC // 128  # n-tiles per core (2)
SLOT = 2 * NT * B   # cols per gather slot: [h0|h1] x [nt2] x [B] = 256

FP = mybir.dt.float32
BF = mybir.dt.bfloat16
AF = mybir.ActivationFunctionType
bf16 = ml_dtypes.bfloat16

_compiled = None


def _build():
    nc = bacc.Bacc("TRN2", target_bir_lowering=False, debug=False, num_devices=NC)

    xt_d = nc.dram_tensor("xt", [EP, T * B], BF, kind="ExternalInput")
    wx0_d = nc.dram_tensor("wx0", [EP, UC], BF, kind="ExternalInput")
    wh0_d = nc.dram_tensor("wh0", [128, NC * NT * NT * 128], BF, kind="ExternalInput")
    wx1_d = nc.dram_tensor("wx1", [128, NC * NT * NT * 128], BF, kind="ExternalInput")
    wh1_d = nc.dram_tensor("wh1", [128, NC * NT * NT * 128], BF, kind="ExternalInput")
    b0_d = nc.dram_tensor("b0", [128, NT], FP, kind="ExternalInput")
    b1_d = nc.dram_tensor("b1", [128, NT], FP, kind="ExternalInput")
    wo_d = nc.dram_tensor("wo", [128, NC * NT], BF, kind="ExternalInput")
    bo_d = nc.dram_tensor("bo", [128, 1], FP, kind="ExternalInput")
    out_d = nc.dram_tensor("out", [B, 1], FP, kind="ExternalOutput")

    with tile.TileContext(nc) as tc:
        with (
            tc.tile_pool(name="const", bufs=1) as const,
            tc.tile_pool(name="state", bufs=1) as state,
            tc.tile_pool(name="chunk", bufs=T + 2) as chunk_pool,
            tc.tile_pool(name="psum", bufs=2, space="PSUM") as psum_pool,
            tc.tile_pool(name="dram", bufs=1, space="DRAM") as dram_pool,
        ):
            # ---- constants ----
            xt = const.tile([EP, T * B], BF)
            wx0 = const.tile([EP, UC], BF)
            wh0 = const.tile([128, NC * NT * NT * 128], BF)
            wx1 = const.tile([128, NC * NT * NT * 128], BF)
            wh1 = const.tile([128, NC * NT * NT * 128], BF)
            b0 = const.tile([128, NT], FP)
            b1 = const.tile([128, NT], FP)
            wo = const.tile([128, NC * NT], BF)
            bo = const.tile([128, 1], FP)
            for sb_t, dr_t in [
                (xt, xt_d), (wx0, wx0_d), (wh0, wh0_d), (wx1, wx1_d),
                (wh1, wh1_d), (b0, b0_d), (b1, b1_d), (wo, wo_d), (bo, bo_d),
            ]:
                nc.sync.dma_start(sb_t[:], dr_t[:])

            # ---- gather buffers ----
            # hg[p][:, s*SLOT + w*NT*B + nt2*B + b]: slot s = chunk of core s.
            hg = [state.tile([128, NC * SLOT], BF, name=f"hg{i}") for i in (0, 1)]
            nc.gpsimd.memset(hg[1][:], 0.0)

            def h_mov(prev_hg, d, w, nt2):
                """Moving operand: h{w}^T k-subtile nt2 of hidden-block d."""
                return prev_hg[:, d * SLOT + (w * NT + nt2) * B:
                               d * SLOT + (w * NT + nt2 + 1) * B]

            def wslice(w_sb, d, nt2, nt):
                i = (d * NT + nt2) * NT + nt
                return w_sb[:, i * 128:(i + 1) * 128]

            def layer_mms(zp, prev_hg, t=None):
                """One layer's matmuls. t given => layer0 (Wx0 x_t + Wh0 h0);
                else Wx1 h0 + Wh1 h1."""
                for nt in range(NT):
                    if t is not None:
                        nc.tensor.matmul(
                            zp[:, nt, :], wx0[:, nt * 128:(nt + 1) * 128],
                            xt[:, t * B:(t + 1) * B], start=True, stop=False)
                        pairs = [(wh0, 0)]
                    else:
                        pairs = [(wx1, 0), (wh1, 1)]
                    n_mm = len(pairs) * NC * NT
                    i = 0
                    for w_sb, w in pairs:
                        for d in range(NC):
                            for nt2 in range(NT):
                                i += 1
                                nc.tensor.matmul(
                                    zp[:, nt, :], wslice(w_sb, d, nt2, nt),
                                    h_mov(prev_hg, d, w, nt2),
                                    start=(t is None and i == 1),
                                    stop=(i == n_mm),
                                )

            def gather(hc_t, t):
                """Gather(t): write my chunk to DRAM, AllGather along the free
                dim into a shared [128, NC*SLOT] buffer, read it back.
                Fresh DRAM tiles per step: shared tiles are single-writer."""
                hcd = dram_pool.tile([128, SLOT], BF, name=f"hcd{t}")
                hgd = dram_pool.tile([NC * 128, SLOT], BF, addr_space="Shared",
                                     name=f"hgd{t}")
                nc.sync.dma_start(hcd[:], hc_t[:])
                nc.gpsimd.collective_compute(
                    "AllGather", mybir.AluOpType.bypass,
                    replica_groups=[list(range(NC))],
                    ins=[hcd[:]], outs=[hgd[:]],
                )
                for s in range(NC):
                    nc.sync.dma_start(hg[t % 2][:, s * SLOT:(s + 1) * SLOT],
                                      hgd[s * 128:(s + 1) * 128, :])

            for t in range(T):
                prev_hg = hg[(t - 1) % 2]
                hc_t = chunk_pool.tile([128, SLOT], BF, tag="hc")

                z0p = psum_pool.tile([128, NT, B], FP, tag="z0")
                layer_mms(z0p, prev_hg, t=t)
                for nt in range(NT):
                    nc.scalar.activation(
                        hc_t[:, nt * B:(nt + 1) * B], z0p[:, nt, :],
                        AF.Tanh, bias=b0[:, nt:nt + 1])

                if t == 0:
                    nc.gpsimd.memset(hc_t[:, NT * B:2 * NT * B], 0.0)
                else:
                    z1p = psum_pool.tile([128, NT, B], FP, tag="z1")
                    layer_mms(z1p, prev_hg)
                    for nt in range(NT):
                        nc.scalar.activation(
                            hc_t[:, (NT + nt) * B:(NT + nt + 1) * B], z1p[:, nt, :],
                            AF.Tanh, bias=b1[:, nt:nt + 1])

                gather(hc_t, t)

            # final h1(T-1): gather only the h1 half into a dedicated buffer
            hc_t = chunk_pool.tile([128, NT * B], BF, tag="hcf")
            z1p = psum_pool.tile([128, NT, B], FP, tag="z1")
            layer_mms(z1p, hg[(T - 1) % 2])
            for nt in range(NT):
                nc.scalar.activation(
                    hc_t[:, nt * B:(nt + 1) * B], z1p[:, nt, :],
                    AF.Tanh, bias=b1[:, nt:nt + 1])
            hgF = state.tile([128, NC * NT * B], BF)
            hcdF = dram_pool.tile([128, NT * B], BF)
            hgdF = dram_pool.tile([NC * 128, NT * B], BF, addr_space="Shared")
            nc.sync.dma_start(hcdF[:], hc_t[:])
            nc.gpsimd.collective_compute(
                "AllGather", mybir.AluOpType.bypass,
                replica_groups=[list(range(NC))],
                ins=[hcdF[:]], outs=[hgdF[:]],
            )
            for s in range(NC):
                nc.sync.dma_start(hgF[:, s * NT * B:(s + 1) * NT * B],
                                  hgdF[s * 128:(s + 1) * 128, :])

            # head: out[b] = sigmoid(sum_k h1[b,k] Wo[k] + bo) on every core
            op = psum_pool.tile([B, 1], FP, tag="head")
            i = 0
            for d in range(NC):
                for nt2 in range(NT):
                    i += 1
                    nc.tensor.matmul(
                        op[:, :], hgF[:, (d * NT + nt2) * B:(d * NT + nt2 + 1) * B],
                        wo[:, d * NT + nt2:d * NT + nt2 + 1],
                        start=(i == 1), stop=(i == NC * NT))
            out_sb = state.tile([B, 1], FP)
            nc.scalar.activation(out_sb[:], op[:], AF.Sigmoid, bias=bo[:B, :])
            nc.sync.dma_start(out_d[:], out_sb[:])

    nc.compile()
    return nc


def _shard_inputs(inputs, emb, Wx0, Wh0, b0, Wx1, Wh1, b1, Wo, bo):
    """Host-side: embed + transpose + per-core slicing (absolute slot order)."""
    x = emb[inputs][:, :T]               # [B, T, E]
    xt = np.ascontiguousarray(x.transpose(2, 1, 0)).reshape(E, T * B)
    xt_p = np.zeros((EP, T * B), bf16)
    xt_p[:E] = xt.astype(bf16)

    def ktile_perm(w, c):
        # [U, UC] col-slice -> [128, 8*2*2*128]; k-tile (d, nt2) holds rows of
        # hidden-block d (absolute — gathered slot d holds core d's chunk).
        wc = w[:, c * UC:(c + 1) * UC].astype(bf16).reshape(NC, NT, 128, NT, 128)
        return np.ascontiguousarray(wc.transpose(2, 0, 1, 3, 4)).reshape(128, -1)

    wo_all = Wo[:, 0].astype(bf16).reshape(NC, NT, 128)

    in_maps = []
    for c in range(NC):
        wx0_c = np.zeros((EP, UC), bf16)
        wx0_c[:E] = Wx0[:, c * UC:(c + 1) * UC].astype(bf16)
        in_maps.append({
            "xt": xt_p,
            "wx0": wx0_c,
            "wh0": ktile_perm(Wh0, c),
            "wx1": ktile_perm(Wx1, c),
            "wh1": ktile_perm(Wh1, c),
            "b0": np.ascontiguousarray(
                b0[c * UC:(c + 1) * UC].reshape(NT, 128).T),
            "b1": np.ascontiguousarray(
                b1[c * UC:(c + 1) * UC].reshape(NT, 128).T),
            "wo": np.ascontiguousarray(wo_all.transpose(2, 0, 1)).reshape(128, -1),
            "bo": np.full((128, 1), bo[0], np.float32),
        })
    return in_maps


def _get_compiled():
    global _compiled
    if _compiled is None:
        _compiled = _build()
    return _compiled


def kernel(inputs, emb, Wx0, Wh0, b0, Wx1, Wh1, b1, Wo, bo, _trace=False,
           _tmpdir=None):
    nc = _get_compiled()
    in_maps = _shard_inputs(
        np.asarray(inputs), np.asarray(emb, np.float32),
        np.asarray(Wx0, np.float32), np.asarray(Wh0, np.float32),
        np.asarray(b0, np.float32), np.asarray(Wx1, np.float32),
        np.asarray(Wh1, np.float32), np.asarray(b1, np.float32),
        np.asarray(Wo, np.float32), np.asarray(bo, np.float32))
    res = bass_utils.run_bass_kernel_spmd(
        nc, in_maps, core_ids=list(range(NC)), trace=_trace, tmpdir=_tmpdir)
    out = res.results[0]["out"]
    if _trace:
        return out, res
    return out


if __name__ == "__main__":
    t0 = time.time()
    _get_compiled()
    print(f"build+compile: {time.time()-t0:.1f}s")


# revision 4
# speedup vs baseline: 1.0199x; 1.0199x over previous
"""Two-layer SimpleRNN (B=64, T=80, U=2048) on 8 TRN2 NeuronCores — v5.

Tensor-parallel like v1 (each core owns a 256-col slice of the hidden units,
weights SBUF-resident, transposed-layout matmuls), but the per-step all-gather
of the [256, 64] h-chunks goes through the runtime collective engine instead
of software-DGE remote DMA: SBUF -> DRAM write, AllGather into a rank-major
shared [NC*128, SLOT] buffer, per-slot DRAM -> SBUF reads.

v1's trace showed each SWDGE remote-DMA prep costs ~7.3us of serialized queue
latency (7 preps/step = ~51us of the 57us step); multi-dest broadcasts and
runtime-branched sends all fail on this HW path. The collective engine is the
one proven alternative (the v1 barrier uses it), needs no per-core constants
(it handles slot offsets internally), no remote semaphores, and no arrival
waits — Tile's ordinary dependency tracking orders write -> CC -> read ->
consuming matmuls.

Receiver slot s holds the chunk of core s (absolute indexing, no XOR
permutation); every core's own chunk also arrives via the CC.

kernel(**inputs) takes the FULL unsharded inputs and returns the FULL output.
"""

import sys
import time

sys.path.insert(0, "/opt/trn_rl_repo")

import numpy as np
import ml_dtypes

import concourse.bass as bass
import concourse.mybir as mybir
import concourse.bacc as bacc
import concourse.tile as tile
import concourse.bass_utils as bass_utils
from concourse.tile_rust import add_dep_helper

B = 64          # batch
import os
T = int(os.environ.get("RNN_T", "80"))  # sequence length
E = 100         # embedding dim
EP = 128        # embedding dim padded to one partition tile
U = 2048        # hidden units
NC = 8          # cores
UC = U // NC    # hidden columns per core (256)
NT = UC // 128  # n-tiles per core (2)
SLOT = 2 * NT * B   # cols per gather slot: [h0|h1] x [nt2] x [B] = 256

FP = mybir.dt.float32
BF = mybir.dt.bfloat16
AF = mybir.ActivationFunctionType
bf16 = ml_dtypes.bfloat16

_compiled = None


def _build():
    nc = bacc.Bacc("TRN2", target_bir_lowering=False, debug=False, num_devices=NC)

    xt_d = nc.dram_tensor("xt", [EP, T * B], BF, kind="ExternalInput")
    wx0_d = nc.dram_tensor("wx0", [EP, UC], BF, kind="ExternalInput")
    wh0_d = nc.dram_tensor("wh0", [128, NC * NT * NT * 128], BF, kind="ExternalInput")
    wx1_d = nc.dram_tensor("wx1", [128, NC * NT * NT * 128], BF, kind="ExternalInput")
    wh1_d = nc.dram_tensor("wh1", [128, NC * NT * NT * 128], BF, kind="ExternalInput")
    b0_d = nc.dram_tensor("b0", [128, NT], FP, kind="ExternalInput")
    b1_d = nc.dram_tensor("b1", [128, NT], FP, kind="ExternalInput")
    wo_d = nc.dram_tensor("wo", [128, NC * NT], BF, kind="ExternalInput")
    bo_d = nc.dram_tensor("bo", [128, 1], FP, kind="ExternalInput")
    out_d = nc.dram_tensor("out", [B, 1], FP, kind="ExternalOutput")

    with tile.TileContext(nc) as tc:
        with (
            tc.tile_pool(name="const", bufs=1) as const,
            tc.tile_pool(name="state", bufs=1) as state,
            tc.tile_pool(name="chunk", bufs=T + 2) as chunk_pool,
            tc.tile_pool(name="psum", bufs=2, space="PSUM") as psum_pool,
            tc.tile_pool(name="dram", bufs=1, space="DRAM") as dram_pool,
        ):
            # ---- constants ----
            xt = const.tile([EP, T * B], BF)
            wx0 = const.tile([EP, UC], BF)
            wh0 = const.tile([128, NC * NT * NT * 128], BF)
            wx1 = const.tile([128, NC * NT * NT * 128], BF)
            wh1 = const.tile([128, NC * NT * NT * 128], BF)
            b0 = const.tile([128, NT], FP)
            b1 = const.tile([128, NT], FP)
            wo = const.tile([128, NC * NT], BF)
            bo = const.tile([128, 1], FP)
            for sb_t, dr_t in [
                (xt, xt_d), (wx0, wx0_d), (wh0, wh0_d), (wx1, wx1_d),
                (wh1, wh1_d), (b0, b0_d), (b1, b1_d), (wo, wo_d), (bo, bo_d),
            ]:
                nc.sync.dma_start(sb_t[:], dr_t[:])

            # ---- gather buffers ----
            # hg[p][:, s*SLOT + w*NT*B + nt2*B + b]: slot s = chunk of core s.
            hg = [state.tile([128, NC * SLOT], BF, name=f"hg{i}") for i in (0, 1)]
            nc.gpsimd.memset(hg[1][:], 0.0)

            def h_mov(prev_hg, d, w, nt2):
                """Moving operand: h{w}^T k-subtile nt2 of hidden-block d."""
                return prev_hg[:, d * SLOT + (w * NT + nt2) * B:
                               d * SLOT + (w * NT + nt2 + 1) * B]

            def wslice(w_sb, d, nt2, nt):
                i = (d * NT + nt2) * NT + nt
                return w_sb[:, i * 128:(i + 1) * 128]

            def layer_mms(zp, prev_hg, t=None):
                """One layer's matmuls. t given => layer0 (Wx0 x_t + Wh0 h0);
                else Wx1 h0 + Wh1 h1."""
                for nt in range(NT):
                    if t is not None:
                        nc.tensor.matmul(
                            zp[:, nt, :], wx0[:, nt * 128:(nt + 1) * 128],
                            xt[:, t * B:(t + 1) * B], start=True, stop=False)
                        pairs = [(wh0, 0)]
                    else:
                        pairs = [(wx1, 0), (wh1, 1)]
                    n_mm = len(pairs) * NC * NT
                    i = 0
                    for w_sb, w in pairs:
                        for d in range(NC):
                            for nt2 in range(NT):
                                i += 1
                                nc.tensor.matmul(
                                    zp[:, nt, :], wslice(w_sb, d, nt2, nt),
                                    h_mov(prev_hg, d, w, nt2),
                                    start=(t is None and i == 1),
                                    stop=(i == n_mm),
                                )

            def gather(hc_t, t):
                """Gather(t): write my chunk to DRAM, AllGather along the free
                dim into a shared [128, NC*SLOT] buffer, read it back.
                Fresh DRAM tiles per step: shared tiles are single-writer."""
                hcd = dram_pool.tile([128, SLOT], BF, name=f"hcd{t}")
                hgd = dram_pool.tile([NC * 128, SLOT], BF, addr_space="Shared",
                                     name=f"hgd{t}")
                nc.sync.dma_start(hcd[:], hc_t[:])
                nc.gpsimd.collective_compute(
                    "AllGather", mybir.AluOpType.bypass,
                    replica_groups=[list(range(NC))],
                    ins=[hcd[:]], outs=[hgd[:]],
                )
                # spread read issues across idle engines (issue ~0.6us each)
                eng = [nc.sync, nc.sync, nc.sync, nc.sync, nc.scalar,
                       nc.scalar, nc.scalar, nc.scalar]
                for s in range(NC):
                    eng[s].dma_start(hg[t % 2][:, s * SLOT:(s + 1) * SLOT],
                                     hgd[s * 128:(s + 1) * 128, :])

            for t in range(T):
                prev_hg = hg[(t - 1) % 2]
                hc_t = chunk_pool.tile([128, SLOT], BF, tag="hc")

                z0p = psum_pool.tile([128, NT, B], FP, tag="z0")
                layer_mms(z0p, prev_hg, t=t)
                for nt in range(NT):
                    nc.scalar.activation(
                        hc_t[:, nt * B:(nt + 1) * B], z0p[:, nt, :],
                        AF.Tanh, bias=b0[:, nt:nt + 1])

                if t == 0:
                    nc.gpsimd.memset(hc_t[:, NT * B:2 * NT * B], 0.0)
                else:
                    z1p = psum_pool.tile([128, NT, B], FP, tag="z1")
                    layer_mms(z1p, prev_hg)
                    for nt in range(NT):
                        nc.scalar.activation(
                            hc_t[:, (NT + nt) * B:(NT + nt + 1) * B], z1p[:, nt, :],
                            AF.Tanh, bias=b1[:, nt:nt + 1])

                gather(hc_t, t)

            # final h1(T-1): gather only the h1 half into a dedicated buffer
            hc_t = chunk_pool.tile([128, NT * B], BF, tag="hcf")
            z1p = psum_pool.tile([128, NT, B], FP, tag="z1")
            layer_mms(z1p, hg[(T - 1) % 2])
            for nt in range(NT):
                nc.scalar.activation(
                    hc_t[:, nt * B:(nt + 1) * B], z1p[:, nt, :],
                    AF.Tanh, bias=b1[:, nt:nt + 1])
            hgF = state.tile([128, NC * NT * B], BF)
            hcdF = dram_pool.tile([128, NT * B], BF)
            hgdF = dram_pool.tile([NC * 128, NT * B], BF, addr_space="Shared")
            nc.sync.dma_start(hcdF[:], hc_t[:])
            nc.gpsimd.collective_compute(
                "AllGather", mybir.AluOpType.bypass,
                replica_groups=[list(range(NC))],
                ins=[hcdF[:]], outs=[hgdF[:]],
            )
            engF = [nc.sync, nc.sync, nc.sync, nc.sync, nc.scalar,
                    nc.scalar, nc.scalar, nc.scalar]
            for s in range(NC):
                engF[s].dma_start(hgF[:, s * NT * B:(s + 1) * NT * B],
                                  hgdF[s * 128:(s + 1) * 128, :])

            # head: out[b] = sigmoid(sum_k h1[b,k] Wo[k] + bo) on every core
            op = psum_pool.tile([B, 1], FP, tag="head")
            i = 0
            for d in range(NC):
                for nt2 in range(NT):
                    i += 1
                    nc.tensor.matmul(
                        op[:, :], hgF[:, (d * NT + nt2) * B:(d * NT + nt2 + 1) * B],
                        wo[:, d * NT + nt2:d * NT + nt2 + 1],
                        start=(i == 1), stop=(i == NC * NT))
            out_sb = state.tile([B, 1], FP)
            nc.scalar.activation(out_sb[:], op[:], AF.Sigmoid, bias=bo[:B, :])
            nc.sync.dma_start(out_d[:], out_sb[:])

    nc.compile()
    return nc


def _shard_inputs(inputs, emb, Wx0, Wh0, b0, Wx1, Wh1, b1, Wo, bo):
    """Host-side: embed + transpose + per-core slicing (absolute slot order)."""
    x = emb[inputs][:, :T]               # [B, T, E]
    xt = np.ascontiguousarray(x.transpose(2, 1, 0)).reshape(E, T * B)
    xt_p = np.zeros((EP, T * B), bf16)
    xt_p[:E] = xt.astype(bf16)

    def ktile_perm(w, c):
        # [U, UC] col-slice -> [128, 8*2*2*128]; k-tile (d, nt2) holds rows of
        # hidden-block d (absolute — gathered slot d holds core d's chunk).
        wc = w[:, c * UC:(c + 1) * UC].astype(bf16).reshape(NC, NT, 128, NT, 128)
        return np.ascontiguousarray(wc.transpose(2, 0, 1, 3, 4)).reshape(128, -1)

    wo_all = Wo[:, 0].astype(bf16).reshape(NC, NT, 128)

    in_maps = []
    for c in range(NC):
        wx0_c = np.zeros((EP, UC), bf16)
        wx0_c[:E] = Wx0[:, c * UC:(c + 1) * UC].astype(bf16)
        in_maps.append({
            "xt": xt_p,
            "wx0": wx0_c,
            "wh0": ktile_perm(Wh0, c),
            "wx1": ktile_perm(Wx1, c),
            "wh1": ktile_perm(Wh1, c),
            "b0": np.ascontiguousarray(
                b0[c * UC:(c + 1) * UC].reshape(NT, 128).T),
            "b1": np.ascontiguousarray(
                b1[c * UC:(c + 1) * UC].reshape(NT, 128).T),
            "wo": np.ascontiguousarray(wo_all.transpose(2, 0, 1)).reshape(128, -1),
            "bo": np.full((128, 1), bo[0], np.float32),
        })
    return in_maps


def _get_compiled():
    global _compiled
    if _compiled is None:
        _compiled = _build()
    return _compiled


def kernel(inputs, emb, Wx0, Wh0, b0, Wx1, Wh1, b1, Wo, bo, _trace=False,
           _tmpdir=None):
    nc = _get_compiled()
    in_maps = _shard_inputs(
        np.asarray(inputs), np.asarray(emb, np.float32),
        np.asarray(Wx0, np.float32), np.asarray(Wh0, np.float32),
        np.asarray(b0, np.float32), np.asarray(Wx1, np.float32),
        np.asarray(Wh1, np.float32), np.asarray(b1, np.float32),
        np.asarray(Wo, np.float32), np.asarray(bo, np.float32))
    res = bass_utils.run_bass_kernel_spmd(
        nc, in_maps, core_ids=list(range(NC)), trace=_trace, tmpdir=_tmpdir)
    out = res.results[0]["out"]
    if _trace:
        return out, res
    return out


if __name__ == "__main__":
    t0 = time.time()
    _get_compiled()
    print(f"build+compile: {time.time()-t0:.1f}s")


# revision 5
# speedup vs baseline: 1.0506x; 1.0300x over previous
"""Two-layer SimpleRNN (B=64, T=80, U=2048) on 8 TRN2 NeuronCores — v5.

Tensor-parallel like v1 (each core owns a 256-col slice of the hidden units,
weights SBUF-resident, transposed-layout matmuls), but the per-step all-gather
of the [256, 64] h-chunks goes through the runtime collective engine instead
of software-DGE remote DMA: SBUF -> DRAM write, AllGather into a rank-major
shared [NC*128, SLOT] buffer, per-slot DRAM -> SBUF reads.

v1's trace showed each SWDGE remote-DMA prep costs ~7.3us of serialized queue
latency (7 preps/step = ~51us of the 57us step); multi-dest broadcasts and
runtime-branched sends all fail on this HW path. The collective engine is the
one proven alternative (the v1 barrier uses it), needs no per-core constants
(it handles slot offsets internally), no remote semaphores, and no arrival
waits — Tile's ordinary dependency tracking orders write -> CC -> read ->
consuming matmuls.

Receiver slot s holds the chunk of core s (absolute indexing, no XOR
permutation); every core's own chunk also arrives via the CC.

kernel(**inputs) takes the FULL unsharded inputs and returns the FULL output.
"""

import sys
import time

sys.path.insert(0, "/opt/trn_rl_repo")

import numpy as np
import ml_dtypes

import concourse.bass as bass
import concourse.mybir as mybir
import concourse.bacc as bacc
import concourse.tile as tile
import concourse.bass_utils as bass_utils
from concourse.tile_rust import add_dep_helper

B = 64          # batch
import os
T = int(os.environ.get("RNN_T", "80"))  # sequence length
E = 100         # embedding dim
EP = 128        # embedding dim padded to one partition tile
U = 2048        # hidden units
NC = 8          # cores
UC = U // NC    # hidden columns per core (256)
NT = UC // 128  # n-tiles per core (2)
SLOT = 2 * NT * B   # cols per gather slot: [h0|h1] x [nt2] x [B] = 256

FP = mybir.dt.float32
BF = mybir.dt.bfloat16
AF = mybir.ActivationFunctionType
bf16 = ml_dtypes.bfloat16

_compiled = None


def _build():
    nc = bacc.Bacc("TRN2", target_bir_lowering=False, debug=False, num_devices=NC)

    xt_d = nc.dram_tensor("xt", [EP, T * B], BF, kind="ExternalInput")
    wx0_d = nc.dram_tensor("wx0", [EP, UC], BF, kind="ExternalInput")
    wh0_d = nc.dram_tensor("wh0", [128, NC * NT * NT * 128], BF, kind="ExternalInput")
    wx1_d = nc.dram_tensor("wx1", [128, NC * NT * NT * 128], BF, kind="ExternalInput")
    wh1_d = nc.dram_tensor("wh1", [128, NC * NT * NT * 128], BF, kind="ExternalInput")
    b0_d = nc.dram_tensor("b0", [128, NT], FP, kind="ExternalInput")
    b1_d = nc.dram_tensor("b1", [128, NT], FP, kind="ExternalInput")
    wo_d = nc.dram_tensor("wo", [128, NC * NT], BF, kind="ExternalInput")
    bo_d = nc.dram_tensor("bo", [128, 1], FP, kind="ExternalInput")
    out_d = nc.dram_tensor("out", [B, 1], FP, kind="ExternalOutput")

    with tile.TileContext(nc) as tc:
        with (
            tc.tile_pool(name="const", bufs=1) as const,
            tc.tile_pool(name="state", bufs=1) as state,
            tc.tile_pool(name="chunk", bufs=T + 2) as chunk_pool,
            tc.tile_pool(name="psum", bufs=2, space="PSUM") as psum_pool,
            tc.tile_pool(name="dram", bufs=1, space="DRAM") as dram_pool,
        ):
            # ---- constants ----
            xt = const.tile([EP, T * B], BF)
            wx0 = const.tile([EP, UC], BF)
            wh0 = const.tile([128, NC * NT * NT * 128], BF)
            wx1 = const.tile([128, NC * NT * NT * 128], BF)
            wh1 = const.tile([128, NC * NT * NT * 128], BF)
            b0 = const.tile([128, NT], FP)
            b1 = const.tile([128, NT], FP)
            wo = const.tile([128, NC * NT], BF)
            bo = const.tile([128, 1], FP)
            for sb_t, dr_t in [
                (xt, xt_d), (wx0, wx0_d), (wh0, wh0_d), (wx1, wx1_d),
                (wh1, wh1_d), (b0, b0_d), (b1, b1_d), (wo, wo_d), (bo, bo_d),
            ]:
                nc.sync.dma_start(sb_t[:], dr_t[:])

            # ---- gather buffers ----
            # hg[p][s][:, w*NT*B + nt2*B + b]: slot s = chunk of core s. One
            # tile per slot so Tile tracks read-DMA -> matmul deps per slot
            # (a single tile serializes all matmuls behind the last read).
            hg = [[state.tile([128, SLOT], BF, name=f"hg{i}_{s}")
                   for s in range(NC)] for i in (0, 1)]
            for i in (0, 1):
                for s in range(NC):
                    nc.gpsimd.memset(hg[i][s][:], 0.0)

            def h_mov(prev_hg, d, w, nt2):
                """Moving operand: h{w}^T k-subtile nt2 of hidden-block d."""
                return prev_hg[d][:, (w * NT + nt2) * B:(w * NT + nt2 + 1) * B]

            def wslice(w_sb, d, nt2, nt):
                i = (d * NT + nt2) * NT + nt
                return w_sb[:, i * 128:(i + 1) * 128]

            def layer_mms(zp, prev_hg, t=None):
                """One layer's matmuls. t given => layer0 (Wx0 x_t + Wh0 h0);
                else Wx1 h0 + Wh1 h1."""
                for nt in range(NT):
                    if t is not None:
                        nc.tensor.matmul(
                            zp[:, nt, :], wx0[:, nt * 128:(nt + 1) * 128],
                            xt[:, t * B:(t + 1) * B], start=True, stop=False)
                        pairs = [(wh0, 0)]
                    else:
                        pairs = [(wx1, 0), (wh1, 1)]
                    n_mm = len(pairs) * NC * NT
                    i = 0
                    for w_sb, w in pairs:
                        for d in range(NC):
                            for nt2 in range(NT):
                                i += 1
                                nc.tensor.matmul(
                                    zp[:, nt, :], wslice(w_sb, d, nt2, nt),
                                    h_mov(prev_hg, d, w, nt2),
                                    start=(t is None and i == 1),
                                    stop=(i == n_mm),
                                )

            def gather(hc_t, t):
                """Gather(t): write my chunk to DRAM, AllGather along the free
                dim into a shared [128, NC*SLOT] buffer, read it back.
                Fresh DRAM tiles per step: shared tiles are single-writer."""
                hcd = dram_pool.tile([128, SLOT], BF, name=f"hcd{t}")
                hgd = dram_pool.tile([NC * 128, SLOT], BF, addr_space="Shared",
                                     name=f"hgd{t}")
                nc.sync.dma_start(hcd[:], hc_t[:])
                nc.gpsimd.collective_compute(
                    "AllGather", mybir.AluOpType.bypass,
                    replica_groups=[list(range(NC))],
                    ins=[hcd[:]], outs=[hgd[:]],
                )
                # spread read issues across idle engines (issue ~0.6us each)
                eng = [nc.sync, nc.sync, nc.sync, nc.sync, nc.scalar,
                       nc.scalar, nc.scalar, nc.scalar]
                for s in range(NC):
                    eng[s].dma_start(hg[t % 2][s][:],
                                     hgd[s * 128:(s + 1) * 128, :])

            for t in range(T):
                prev_hg = hg[(t - 1) % 2]
                hc_t = chunk_pool.tile([128, SLOT], BF, tag="hc")

                z0p = psum_pool.tile([128, NT, B], FP, tag="z0")
                layer_mms(z0p, prev_hg, t=t)
                for nt in range(NT):
                    nc.scalar.activation(
                        hc_t[:, nt * B:(nt + 1) * B], z0p[:, nt, :],
                        AF.Tanh, bias=b0[:, nt:nt + 1])

                if t == 0:
                    nc.gpsimd.memset(hc_t[:, NT * B:2 * NT * B], 0.0)
                else:
                    z1p = psum_pool.tile([128, NT, B], FP, tag="z1")
                    layer_mms(z1p, prev_hg)
                    for nt in range(NT):
                        nc.scalar.activation(
                            hc_t[:, (NT + nt) * B:(NT + nt + 1) * B], z1p[:, nt, :],
                            AF.Tanh, bias=b1[:, nt:nt + 1])

                gather(hc_t, t)

            # final h1(T-1): gather only the h1 half into a dedicated buffer
            hc_t = chunk_pool.tile([128, NT * B], BF, tag="hcf")
            z1p = psum_pool.tile([128, NT, B], FP, tag="z1")
            layer_mms(z1p, hg[(T - 1) % 2])
            for nt in range(NT):
                nc.scalar.activation(
                    hc_t[:, nt * B:(nt + 1) * B], z1p[:, nt, :],
                    AF.Tanh, bias=b1[:, nt:nt + 1])
            hgF = state.tile([128, NC * NT * B], BF)
            hcdF = dram_pool.tile([128, NT * B], BF)
            hgdF = dram_pool.tile([NC * 128, NT * B], BF, addr_space="Shared")
            nc.sync.dma_start(hcdF[:], hc_t[:])
            nc.gpsimd.collective_compute(
                "AllGather", mybir.AluOpType.bypass,
                replica_groups=[list(range(NC))],
                ins=[hcdF[:]], outs=[hgdF[:]],
            )
            engF = [nc.sync, nc.sync, nc.sync, nc.sync, nc.scalar,
                    nc.scalar, nc.scalar, nc.scalar]
            for s in range(NC):
                engF[s].dma_start(hgF[:, s * NT * B:(s + 1) * NT * B],
                                  hgdF[s * 128:(s + 1) * 128, :])

            # head: out[b] = sigmoid(sum_k h1[b,k] Wo[k] + bo) on every core
            op = psum_pool.tile([B, 1], FP, tag="head")
            i = 0
            for d in range(NC):
                for nt2 in range(NT):
                    i += 1
                    nc.tensor.matmul(
                        op[:, :], hgF[:, (d * NT + nt2) * B:(d * NT + nt2 + 1) * B],
                        wo[:, d * NT + nt2:d * NT + nt2 + 1],
                        start=(i == 1), stop=(i == NC * NT))
            out_sb = state.tile([B, 1], FP)
            nc.scalar.activation(out_sb[:], op[:], AF.Sigmoid, bias=bo[:B, :])
            nc.sync.dma_start(out_d[:], out_sb[:])

    nc.compile()
    return nc


def _shard_inputs(inputs, emb, Wx0, Wh0, b0, Wx1, Wh1, b1, Wo, bo):
    """Host-side: embed + transpose + per-core slicing (absolute slot order)."""
    x = emb[inputs][:, :T]               # [B, T, E]
    xt = np.ascontiguousarray(x.transpose(2, 1, 0)).reshape(E, T * B)
    xt_p = np.zeros((EP, T * B), bf16)
    xt_p[:E] = xt.astype(bf16)

    def ktile_perm(w, c):
        # [U, UC] col-slice -> [128, 8*2*2*128]; k-tile (d, nt2) holds rows of
        # hidden-block d (absolute — gathered slot d holds core d's chunk).
        wc = w[:, c * UC:(c + 1) * UC].astype(bf16).reshape(NC, NT, 128, NT, 128)
        return np.ascontiguousarray(wc.transpose(2, 0, 1, 3, 4)).reshape(128, -1)

    wo_all = Wo[:, 0].astype(bf16).reshape(NC, NT, 128)

    in_maps = []
    for c in range(NC):
        wx0_c = np.zeros((EP, UC), bf16)
        wx0_c[:E] = Wx0[:, c * UC:(c + 1) * UC].astype(bf16)
        in_maps.append({
            "xt": xt_p,
            "wx0": wx0_c,
            "wh0": ktile_perm(Wh0, c),
            "wx1": ktile_perm(Wx1, c),
            "wh1": ktile_perm(Wh1, c),
            "b0": np.ascontiguousarray(
                b0[c * UC:(c + 1) * UC].reshape(NT, 128).T),
            "b1": np.ascontiguousarray(
                b1[c * UC:(c + 1) * UC].reshape(NT, 128).T),
            "wo": np.ascontiguousarray(wo_all.transpose(2, 0, 1)).reshape(128, -1),
            "bo": np.full((128, 1), bo[0], np.float32),
        })
    return in_maps


def _get_compiled():
    global _compiled
    if _compiled is None:
        _compiled = _build()
    return _compiled


def kernel(inputs, emb, Wx0, Wh0, b0, Wx1, Wh1, b1, Wo, bo, _trace=False,
           _tmpdir=None):
    nc = _get_compiled()
    in_maps = _shard_inputs(
        np.asarray(inputs), np.asarray(emb, np.float32),
        np.asarray(Wx0, np.float32), np.asarray(Wh0, np.float32),
        np.asarray(b0, np.float32), np.asarray(Wx1, np.float32),
        np.asarray(Wh1, np.float32), np.asarray(b1, np.float32),
        np.asarray(Wo, np.float32), np.asarray(bo, np.float32))
    res = bass_utils.run_bass_kernel_spmd(
        nc, in_maps, core_ids=list(range(NC)), trace=_trace, tmpdir=_tmpdir)
    out = res.results[0]["out"]
    if _trace:
        return out, res
    return out


if __name__ == "__main__":
    t0 = time.time()
    _get_compiled()
    print(f"build+compile: {time.time()-t0:.1f}s")
